# revision 13
# baseline (speedup 1.0000x reference)
"""Multi-head attention (B=4, T=2048, D=1024, H=16, causal) on 8 trn2 NeuronCores.

Sharding: core c handles batch b = c//2 and head-group g = c%2 (8 heads,
512 model dims). Q/K/V projections are computed per-core for the core's
head slice (W_q/W_k/W_v column-sharded), attention runs fully on-core,
the output projection uses W_o row-sharded. Each core DMAs its full-D
partial out^T (its head-group's contribution, with b_o/2 folded in) and
the host sums the two partials per batch -- no on-device collective,
which removes the init barrier and the ~30us ReduceScatter tail.

Matmul operands are bf16 (PSUM accumulation stays fp32): halves HBM/SBUF
traffic and eases the PE power throttle that full-rate fp32 trips, at
rel-err ~4e-3 (tolerance 2e-2). Activations are kept transposed
([d, tokens]) on-chip so every matmul operand is naturally K-major:
    Q^T = Wq^T.T @ X^T           (per 128-d' tile, accumulated in PSUM)
    S^T[k,q] = (K^T slice).T @ Q^T slice      (contraction d_k = 64)
    P^T = exp(S^T / 8) with the strict-upper-triangle masked: fully-masked
        column ranges are simply skipped by the PV accumulation, the
        128x128 diagonal block is masked by a 0/1 multiply on GpSimd.
        Diagonal-region tiles are packed width-trimmed in pairs into one
        2-bank PSUM span so one exp instruction covers both (ACT insts
        are ~293ns fixed + 0.83ns/col, so fewer/wider is cheaper).
    [x^T | s] = V_aug.T @ P^T    (V augmented with a ones column -> row sums)
    x^T normalized by s via recip (ACT ln/exp or DVE) + PE outer-product
        replicate + one DVE mul straight from the two PSUM operands
    out^T partial = Wo^T.T @ x^T, + b_o/2 via DVE tensor_scalar_add
        (bias adds live on DVE, keeping ACT exp-only: ACT paces the
        attention phase), DMA straight to outT.
"""
import os
import numpy as np
from contextlib import ExitStack

import concourse.bass as bass
import concourse.tile as tile
import concourse.mybir as mybir
from concourse.bass_utils import run_bass_kernel_spmd
from bass_rust import ScopedClock

f32 = mybir.dt.float32
f32r = mybir.dt.float32r
bf16 = mybir.dt.bfloat16
EXPF = mybir.ActivationFunctionType.Exp
LNF = mybir.ActivationFunctionType.Ln
IDENT = mybir.ActivationFunctionType.Identity

B, T, D = 4, 2048, 1024
H, DK = 16, 64
N_CORES = 8
HPC = 8            # heads per core
DH = HPC * DK      # 512, model dims per core
NEG = -1.0e9

_MODE_MAP = {"sem-ge-imm": "sem-ge", "sem-eq-imm": "sem-eq", "sem-le-imm": "sem-le"}


def _patched_drain_and_barrier(self, tick_clock, wait_clock):
    # This walrus build rejects Drain/NoOp instructions that carry sync
    # waits ("Too many sync wait commands"), which the stock Tile tail
    # emits. Put the tail waits on individual EventSemaphore instructions
    # and use sem-only barriers instead of the drain butterfly.
    nc = self.nc
    collector = nc.sync.nop(nofuse=True, hint="tile_tail_wait")
    wait_clock.add_sem_waits(collector.ins, ScopedClock({None: tick_clock.global_clock}))
    si = collector.ins.sync_info
    waits = list(si.on_wait) if si else []
    if si:
        collector.ins.sync_info = mybir.SyncInfo(on_wait=[], on_update=[])
    assert self.sems is not None
    name2sem = {s.name: s for s in self.sems.allocated().values()}
    for w in waits:
        nc.sync.wait_op(name2sem[w.ant_name], w.wait_value, _MODE_MAP.get(w.wait_mode, "sem-ge"))
    nc.all_engine_barrier(sem_only=True)
    popped = nc._tile_sem_poison_stack.pop()
    assert popped is self._sem_poison
    nc.clear_and_free_semaphores(list(self.sems.allocated().values()))
    nc.all_engine_barrier(sem_only=True)


tile.TileContext._drain_and_barrier = _patched_drain_and_barrier


def _fixup_sync_waits(nc):
    """This walrus build accepts at most 1 sync wait per compute/DMA
    instruction (EventSemaphore: 2). Tile's add_semaphores can emit more.
    Hoist excess waits onto EventSemaphore instructions inserted just
    before the over-budget instruction on the same engine."""
    for bb in nc.main_func.blocks:
        insts = bb.instructions
        out = []
        changed = False
        for ins in insts:
            si = ins.sync_info
            cap = 2 if type(ins).__name__ == "InstEventSemaphore" else 1
            if si is not None and len(si.on_wait) > cap:
                waits = list(si.on_wait)
                keep, excess = waits[-1:], waits[:-1]
                for i in range(0, len(excess), 2):
                    ev = mybir.InstEventSemaphore(
                        name=nc.get_next_instruction_name(),
                        ins=[], outs=[],
                        sync_info=mybir.SyncInfo(on_wait=excess[i:i + 2], on_update=[]),
                    )
                    ev.engine = ins.engine
                    out.append(ev)
                ins.sync_info = mybir.SyncInfo(on_wait=keep, on_update=list(si.on_update))
                changed = True
            out.append(ins)
        if changed:
            bb.instructions = out


def _emit_kernel(nc):
    qT = nc.dram_tensor("qT", [D, T], bf16, kind="ExternalInput")
    kT = nc.dram_tensor("kT", [D, T], bf16, kind="ExternalInput")
    vT = nc.dram_tensor("vT", [D, T], bf16, kind="ExternalInput")
    wq = nc.dram_tensor("wqT", [D, DH], bf16, kind="ExternalInput")
    wk = nc.dram_tensor("wkT", [D, DH], bf16, kind="ExternalInput")
    wv = nc.dram_tensor("wvT", [D, DH], bf16, kind="ExternalInput")
    wo = nc.dram_tensor("woT", [DH, D], bf16, kind="ExternalInput")
    bq = nc.dram_tensor("bq", [128, 4], f32, kind="ExternalInput")
    bk = nc.dram_tensor("bk", [128, 4], f32, kind="ExternalInput")
    bv = nc.dram_tensor("bv", [128, DH], f32, kind="ExternalInput")
    bo = nc.dram_tensor("bo", [128, 8], f32, kind="ExternalInput")
    tri = nc.dram_tensor("tri", [128, 128], bf16, kind="ExternalInput")
    # Each core outputs its head-group's full-D partial of its batch's
    # out^T (b_o/2 included); the host sums the two partials per batch.
    outT = nc.dram_tensor("outT", [D, T], bf16, kind="ExternalOutput")

    with tile.TileContext(nc, num_cores=N_CORES) as tc, ExitStack() as ctx:
        const = ctx.enter_context(tc.tile_pool(name="const", bufs=1))
        perm = ctx.enter_context(tc.tile_pool(name="perm", bufs=1))

        # Persistent on-chip tensors: [p, i, t] = full[i*128+p, t]
        QT = perm.tile([128, 4, T], bf16)
        KT = perm.tile([128, 4, T], bf16)
        Vg = perm.tile([128, 16, HPC * 65], bf16)   # V_aug: per k-tile, 8 heads x (64 vals + 1 one)
        xT = perm.tile([128, 4, T], bf16)

        bq_t = const.tile([128, 4], f32)
        bk_t = const.tile([128, 4], f32)
        bv_t = const.tile([128, DH], f32)
        bo_t = const.tile([128, 8], f32)
        tri_t = const.tile([128, 128], bf16)
        ones_t = const.tile([65, 64], f32r)
        nc.gpsimd.memset(ones_t[:].bitcast(f32), 1.0)
        nc.sync.dma_start(bq_t[:], bq[:])
        nc.sync.dma_start(bk_t[:], bk[:])
        nc.sync.dma_start(bv_t[:], bv[:])
        nc.sync.dma_start(bo_t[:], bo[:])
        nc.sync.dma_start(tri_t[:], tri[:])
        # ones column of V_aug, written once (columns 64 + 65*n, uniform stride)
        nc.gpsimd.memset(Vg[:].rearrange("p i (h j) -> p (i h) j", j=65)[:, :, 64:65], 1.0)

        # PE warmup: dependency-free matmuls that fill the initial input-DMA
        # wait so the HAM clock gate is released before the real work starts.
        with tc.tile_pool(name="warm", bufs=1) as warm, \
                tc.tile_pool(name="warm_psum", bufs=2, space="PSUM") as warm_psum:
            wrm = warm.tile([64, 512], f32r)
            nc.gpsimd.memset(wrm[:].bitcast(f32), 0.0)
            for _ in range(14):
                wp = warm_psum.tile([64, 512], f32)
                nc.tensor.matmul(wp[:], ones_t[0:64, :], wrm[:])

        # ---------------- fused projection + attention stream ----------------
        # The Q/K/V projections are woven INTO the attention pair-block loop:
        # each (hi, qc) iteration first projects exactly the Q/K e-block the
        # upcoming S-pair needs (Q tck=qc e=hi, K tck=qc e=hi), the V chunk
        # for token group qc is projected at hi==0, and the output projection
        # for group qc-1 lands at hi==1. This starts the ACT exp stream at
        # ~25us instead of after a ~105us projection prologue, and keeps
        # every engine's queue primed end-to-end.
        #
        # Heads are processed in PAIRS (2*hi, 2*hi+1): the even head's K/Q
        # rows live in SBUF partitions 0:64, the odd head's in 64:128, so
        # their S matmuls (contraction d_k=64) target disjoint PE row-groups
        # (tile_position auto-derives from base_partition) and run
        # CONCURRENTLY when issued interleaved -- 2x S throughput where a
        # lone d_k=64 matmul leaves half the array idle.
        with ExitStack() as ph:
            wpool = ph.enter_context(tc.tile_pool(name="wproj", bufs=3))
            wopool = ph.enter_context(tc.tile_pool(name="wopool", bufs=1))
            xpool = ph.enter_context(tc.tile_pool(name="xchunk", bufs=5))
            opool = ph.enter_context(tc.tile_pool(name="opool", bufs=3))
            ppool = ph.enter_context(tc.tile_pool(name="ppool", bufs=20))
            rpool = ph.enter_context(tc.tile_pool(name="rpool", bufs=4))
            s_psum = ph.enter_context(tc.tile_pool(name="s_psum", bufs=2, space="PSUM"))
            op_psum = ph.enter_context(tc.tile_pool(name="op_psum", bufs=2, space="PSUM"))
            pv_psum = ph.enter_context(tc.tile_pool(name="pv_psum", bufs=2, space="PSUM"))

            # all weights prefetched up front (DMA overlaps the warmup)
            wqt = wpool.tile([128, 8, DH], bf16, tag="wproj")
            wkt = wpool.tile([128, 8, DH], bf16, tag="wproj")
            wvt = wpool.tile([128, 8, DH], bf16, tag="wproj")
            for wt_, wdram in ((wqt, wq), (wkt, wk), (wvt, wv)):
                wsrc = wdram.rearrange("(i p) n -> p i n", p=128)
                for kt in range(8):
                    nc.sync.dma_start(wt_[:, kt, :], wsrc[:, kt, :])
            wot = wopool.tile([128, 4, D], bf16)
            wosrc = wo.rearrange("(i p) n -> p i n", p=128)
            for kt in range(4):
                nc.sync.dma_start(wot[:, kt, :], wosrc[:, kt, :])

            qsrc = qT.rearrange("(i p) t -> p i t", p=128)
            ksrc = kT.rearrange("(i p) t -> p i t", p=128)
            vsrc = vT.rearrange("(i p) t -> p i t", p=128)
            bv3 = bv_t[:].rearrange("p (h j) -> p h j", h=HPC)

            def stage_chunk(src, tck):
                xc = xpool.tile([128, 8, 512], bf16, tag="xchunk")
                for kt in range(8):
                    nc.sync.dma_start(xc[:, kt, :], src[:, kt, tck * 512:(tck + 1) * 512])
                return xc

            def emit_qk_group(xc, wt_, dst, bias, tck, e):
                ps = op_psum.tile([128, 512], f32, tag="ops")
                for kt in range(8):
                    nc.tensor.matmul(
                        ps[:],
                        wt_[:, kt, e * 128:(e + 1) * 128],
                        xc[:, kt, :],
                        start=(kt == 0), stop=(kt == 7),
                    )
                nc.vector.tensor_scalar_add(
                    dst[:, e, tck * 512:(tck + 1) * 512], ps[:],
                    bias[:, e:e + 1],
                )

            def emit_v_chunk(tg, vxc):
                for tt in range(4):
                    ps = op_psum.tile([128, DH], f32, tag="ops")
                    for kt in range(8):
                        nc.tensor.matmul(
                            ps[:],
                            vxc[:, kt, tt * 128:(tt + 1) * 128],
                            wvt[:, kt, :],
                            start=(kt == 0), stop=(kt == 7),
                        )
                    ti = tg * 4 + tt
                    nc.vector.tensor_add(
                        Vg[:, ti, :].rearrange("p (h j) -> p h j", h=HPC)[:, :, 0:64],
                        ps[:].rearrange("p (h j) -> p h j", h=HPC),
                        bv3,
                    )

            def emit_outproj(tck):
                for e in range(8):
                    ps = op_psum.tile([128, 512], f32, tag="ops")
                    for kt in range(4):
                        nc.tensor.matmul(
                            ps[:],
                            wot[:, kt, e * 128:(e + 1) * 128],
                            xT[:, kt, tck * 512:(tck + 1) * 512],
                            start=(kt == 0), stop=(kt == 3),
                        )
                    ot = opool.tile([128, 512], bf16, tag="otile")
                    nc.vector.tensor_scalar_add(ot[:], ps[:], bo_t[:, e:e + 1])
                    nc.sync.dma_start(
                        outT[e * 128:(e + 1) * 128, tck * 512:(tck + 1) * 512],
                        ot[:],
                    )

            def emit_normalize(pv, rr, po, hi, q0):
                # Replicate 1/d across 64 partitions via PE outer product
                # (this walrus build lacks gpsimd partition_broadcast), then
                # DVE copy + mul (DVE reads only one PSUM operand per inst).
                rp = op_psum.tile([64, 512], f32, tag="ops")
                nc.tensor.matmul(rp[:], ones_t[64:65, :], rr[64:65, :])
                nc.vector.tensor_copy(xT[po:po + 64, hi, q0:q0 + 512], pv[0:64, :])
                nc.vector.tensor_mul(
                    xT[po:po + 64, hi, q0:q0 + 512],
                    xT[po:po + 64, hi, q0:q0 + 512],
                    rp[:],
                )

            def emit_pv(ptiles, h, qc):
                po = 64 * (h % 2)
                hi = h // 2
                q0 = qc * 512
                nkt = 4 * qc + 4
                pv = pv_psum.tile([65, 512], f32)
                for kt in range(nkt):
                    pt, off, vs, w = ptiles[kt]
                    nc.tensor.matmul(
                        pv[:, vs:512],
                        Vg[:, kt, 65 * h:65 * (h + 1)],
                        pt[:, off:off + w],
                        start=(kt == 0), stop=(kt == nkt - 1),
                    )
                rr = rpool.tile([65, 512], f32r, tag="rrow")
                with nc.allow_low_precision(reason="softmax denom recip in f32r"):
                    if qc < 2 or h % 2 == 1:
                        # ACT ln/exp reciprocal (~1.4us, shares the exp table
                        # set) where the DVE lane-serial reciprocal (~3.4us)
                        # would pace the block; split so neither engine
                        # saturates.
                        srl = rpool.tile([1, 512], f32, tag="srl")
                        nc.scalar.activation(srl[:], pv[64:65, :], LNF)
                        nc.scalar.activation(rr[64:65, :], srl[:], EXPF, scale=-1.0)
                    else:
                        nc.vector.reciprocal(rr[64:65, :], pv[64:65, :])
                return (pv, rr, po, hi, q0)

            def emit_spairs(hi, qc):
                """S + exp for the head pair (2*hi, 2*hi+1), matmuls issued
                alternating row-halves so the two heads' S tiles run
                concurrently in the PE array."""
                q0 = qc * 512
                ptA, ptB = [], []
                # fully-allowed tiles (kt < 4*qc): per-head [128,1024] spans
                # (2 k-tiles), one exp instruction per span
                for m in range(2 * qc):
                    spA = s_psum.tile([128, 1024], f32, tag="spair")
                    ptA2 = ppool.tile([128, 1024], bf16, tag="ppair", bufs=8)
                    spB = s_psum.tile([128, 1024], f32, tag="spair")
                    ptB2 = ppool.tile([128, 1024], bf16, tag="ppair", bufs=8)
                    for half in range(2):
                        kt = 2 * m + half
                        c0 = half * 512
                        nc.tensor.matmul(
                            spA[:, c0:c0 + 512],
                            KT[0:64, hi, kt * 128:(kt + 1) * 128],
                            QT[0:64, hi, q0:q0 + 512],
                        )
                        nc.tensor.matmul(
                            spB[:, c0:c0 + 512],
                            KT[64:128, hi, kt * 128:(kt + 1) * 128],
                            QT[64:128, hi, q0:q0 + 512],
                        )
                    nc.scalar.activation(ptA2[:], spA[:], EXPF, scale=0.125)
                    nc.scalar.activation(ptB2[:], spB[:], EXPF, scale=0.125)
                    ptA.append((ptA2, 0, 0, 512))
                    ptA.append((ptA2, 512, 0, 512))
                    ptB.append((ptB2, 0, 0, 512))
                    ptB.append((ptB2, 512, 0, 512))
                # diagonal-region tiles, width-trimmed pairs (512|384) and
                # (256|128) packed per head into one [128,1024] PSUM span
                for dp in range(2):
                    spA = s_psum.tile([128, 1024], f32, tag="spair")
                    ptA2 = ppool.tile([128, 1024], bf16, tag="ppair", bufs=8)
                    spB = s_psum.tile([128, 1024], f32, tag="spair")
                    ptB2 = ppool.tile([128, 1024], bf16, tag="ppair", bufs=8)
                    pair_off = 0
                    for half in range(2):
                        j = 2 * dp + half
                        kt = 4 * qc + j
                        vs = j * 128
                        w = 512 - vs
                        nc.tensor.matmul(
                            spA[:, pair_off:pair_off + w],
                            KT[0:64, hi, kt * 128:(kt + 1) * 128],
                            QT[0:64, hi, q0 + vs:q0 + 512],
                        )
                        nc.tensor.matmul(
                            spB[:, pair_off:pair_off + w],
                            KT[64:128, hi, kt * 128:(kt + 1) * 128],
                            QT[64:128, hi, q0 + vs:q0 + 512],
                        )
                        ptA.append((ptA2, pair_off, vs, w))
                        ptB.append((ptB2, pair_off, vs, w))
                        pair_off += w
                    nc.scalar.activation(
                        ptA2[:, 0:pair_off], spA[:, 0:pair_off],
                        EXPF, scale=0.125,
                    )
                    nc.scalar.activation(
                        ptB2[:, 0:pair_off], spB[:, 0:pair_off],
                        EXPF, scale=0.125,
                    )
                    for half in range(2):
                        off = ptA[-2 + half][1]
                        nc.gpsimd.tensor_mul(
                            ptA2[:, off:off + 128], ptA2[:, off:off + 128],
                            tri_t[:],
                        )
                        nc.gpsimd.tensor_mul(
                            ptB2[:, off:off + 128], ptB2[:, off:off + 128],
                            tri_t[:],
                        )
                return ptA, ptB

            # Two-deep software pipeline over the 16 (hi, qc) pair-blocks:
            # per iteration [Q/K proj group] [PV pair(i-1) + norms(i-2)]
            # [S(i)], so the PE never waits on the exps (ACT) of the block
            # it just produced and projections hide inside the stream.
            pending_pv = None
            pending_nm = None
            pending_op = None
            for qc in range(4):
                qxc = stage_chunk(qsrc, qc)
                kxc = stage_chunk(ksrc, qc)
                vxc = stage_chunk(vsrc, qc)
                for hi in range(4):
                    emit_qk_group(qxc, wqt, QT, bq_t, qc, hi)
                    emit_qk_group(kxc, wkt, KT, bk_t, qc, hi)
                    if pending_pv is not None:
                        pA, pB, phi, pqc = pending_pv
                        nmA = emit_pv(pA, 2 * phi, pqc)
                        nmB = emit_pv(pB, 2 * phi + 1, pqc)
                        if pending_nm is not None:
                            for nm in pending_nm:
                                emit_normalize(*nm)
                        pending_nm = [nmA, nmB]
                        pending_pv = None
                    ptAB = emit_spairs(hi, qc)
                    if hi == 0:
                        emit_v_chunk(qc, vxc)
                    if hi == 1 and pending_op is not None:
                        emit_outproj(pending_op)
                        pending_op = None
                    pending_pv = ptAB + (hi, qc)

                # group flush: finish every block of this qc so its xT columns
                # are final; the outproj emission is deferred into qc+1
                pA, pB, phi, pqc = pending_pv
                nmA = emit_pv(pA, 2 * phi, pqc)
                nmB = emit_pv(pB, 2 * phi + 1, pqc)
                pending_pv = None
                if pending_nm is not None:
                    for nm in pending_nm:
                        emit_normalize(*nm)
                emit_normalize(*nmA)
                emit_normalize(*nmB)
                pending_nm = None
                pending_op = qc
            emit_outproj(pending_op)

_NC_CACHE = None


def _build_nc():
    global _NC_CACHE
    if _NC_CACHE is None:
        nc = bass.Bass("TRN2", target_bir_lowering=False, debug=False, num_devices=N_CORES)
        _emit_kernel(nc)
        _fixup_sync_waits(nc)
        _NC_CACHE = nc
    return _NC_CACHE


def _host_mask_tiles(attention_mask, key_padding_mask):
    # The kernel exploits the causal structure; verify the runtime masks
    # actually match it (they do for this problem's setup_inputs()).
    am = np.asarray(attention_mask)[0]
    causal = np.triu(np.ones((T, T), np.int32), k=1)
    if not np.array_equal(am != 0, causal != 0):
        raise ValueError("kernel specialised for strict-upper-triangular causal mask")
    if np.asarray(key_padding_mask).any():
        raise ValueError("kernel specialised for all-attendable key_padding_mask")
    # 0/1 multiplicative mask for the 128x128 diagonal block of P^T[k, q]:
    # allowed iff q >= k.
    dk = np.arange(128)[:, None]
    dq = np.arange(128)[None, :]
    return (dq >= dk).astype(np.float32)


def _make_in_maps(inputs):
    import ml_dtypes
    bf = ml_dtypes.bfloat16
    query = np.asarray(inputs["query"], np.float32).astype(bf)
    key = np.asarray(inputs["key"], np.float32).astype(bf)
    value = np.asarray(inputs["value"], np.float32).astype(bf)
    W = {n: np.asarray(inputs[n], np.float32).astype(bf)
         for n in ("W_q", "W_k", "W_v", "W_o")}
    b = {n: np.asarray(inputs[n], np.float32) for n in ("b_q", "b_k", "b_v", "b_o")}
    msk = _host_mask_tiles(inputs["attention_mask"], inputs["key_padding_mask"])

    in_maps = []
    for c in range(N_CORES):
        bb, g = c // 2, c % 2
        hsel = slice(DH * g, DH * (g + 1))
        in_maps.append({
            "qT": np.ascontiguousarray(query[bb].T),
            "kT": np.ascontiguousarray(key[bb].T),
            "vT": np.ascontiguousarray(value[bb].T),
            "wqT": np.ascontiguousarray(W["W_q"].T[:, hsel]),
            "wkT": np.ascontiguousarray(W["W_k"].T[:, hsel]),
            "wvT": np.ascontiguousarray(W["W_v"].T[:, hsel]),
            "woT": np.ascontiguousarray(W["W_o"].T[hsel, :]),
            "bq": np.ascontiguousarray(b["b_q"][hsel].reshape(4, 128).T),
            "bk": np.ascontiguousarray(b["b_k"][hsel].reshape(4, 128).T),
            "bv": np.tile(b["b_v"][hsel][None, :], (128, 1)),
            "bo": np.ascontiguousarray((0.5 * b["b_o"]).reshape(8, 128).T),
            "tri": msk.astype(bf),
        })
    return in_maps


def assemble_output(results):
    """Sum the two per-head-group partial out^T per batch (host-side
    unshard: each core's outT is its head-group's full-D partial)."""
    out = np.empty((B, T, D), np.float32)
    for bb in range(B):
        p0 = np.asarray(results[2 * bb]["outT"], np.float32)
        p1 = np.asarray(results[2 * bb + 1]["outT"], np.float32)
        out[bb] = (p0 + p1).T
    return out


def kernel(**inputs):
    nc = _build_nc()
    in_maps = _make_in_maps(inputs)
    out = None
    for _attempt in range(3):
        res = run_bass_kernel_spmd(nc, in_maps, core_ids=list(range(N_CORES)))
        out = assemble_output(res.results)
        # rare transient device flake can corrupt an execution; retry
        if np.isfinite(out).all():
            break
    return out



# revision 15
# speedup vs baseline: 1.1010x; 1.1010x over previous
"""Multi-head attention (B=4, T=2048, D=1024, H=16, causal) on 8 trn2 NeuronCores.

Sharding: core c handles batch b = c//2 and head-group g = c%2 (8 heads,
512 model dims). Q/K/V projections are computed per-core for the core's
head slice (W_q/W_k/W_v column-sharded), attention runs fully on-core,
the output projection uses W_o row-sharded. Each core DMAs its full-D
partial out^T (its head-group's contribution, with b_o/2 folded in) and
the host sums the two partials per batch -- no on-device collective,
which removes the init barrier and the ~30us ReduceScatter tail.

Matmul operands are bf16 (PSUM accumulation stays fp32): halves HBM/SBUF
traffic and eases the PE power throttle that full-rate fp32 trips, at
rel-err ~4e-3 (tolerance 2e-2). Activations are kept transposed
([d, tokens]) on-chip so every matmul operand is naturally K-major:
    Q^T = Wq^T.T @ X^T           (per 128-d' tile, accumulated in PSUM)
    S^T[k,q] = (K^T slice).T @ Q^T slice      (contraction d_k = 64)
    P^T = exp(S^T / 8) with the strict-upper-triangle masked: fully-masked
        column ranges are simply skipped by the PV accumulation, the
        128x128 diagonal block is masked by a 0/1 multiply on GpSimd.
        Diagonal-region tiles are packed width-trimmed in pairs into one
        2-bank PSUM span so one exp instruction covers both (ACT insts
        are ~293ns fixed + 0.83ns/col, so fewer/wider is cheaper).
    [x^T | s] = V_aug.T @ P^T    (V augmented with a ones column -> row sums)
    x^T normalized by s via recip (ACT ln/exp or DVE) + PE outer-product
        replicate + one DVE mul straight from the two PSUM operands
    out^T partial = Wo^T.T @ x^T, + b_o/2 via DVE tensor_scalar_add
        (bias adds live on DVE, keeping ACT exp-only: ACT paces the
        attention phase), DMA straight to outT.
"""
import os
import numpy as np
from contextlib import ExitStack

import concourse.bass as bass
import concourse.tile as tile
import concourse.mybir as mybir
from concourse.bass_utils import run_bass_kernel_spmd
from bass_rust import ScopedClock

f32 = mybir.dt.float32
f32r = mybir.dt.float32r
bf16 = mybir.dt.bfloat16
EXPF = mybir.ActivationFunctionType.Exp
LNF = mybir.ActivationFunctionType.Ln
IDENT = mybir.ActivationFunctionType.Identity

B, T, D = 4, 2048, 1024
H, DK = 16, 64
N_CORES = 8
HPC = 8            # heads per core
DH = HPC * DK      # 512, model dims per core
NEG = -1.0e9

_MODE_MAP = {"sem-ge-imm": "sem-ge", "sem-eq-imm": "sem-eq", "sem-le-imm": "sem-le"}


def _patched_drain_and_barrier(self, tick_clock, wait_clock):
    # This walrus build rejects Drain/NoOp instructions that carry sync
    # waits ("Too many sync wait commands"), which the stock Tile tail
    # emits. Put the tail waits on individual EventSemaphore instructions
    # and use sem-only barriers instead of the drain butterfly.
    nc = self.nc
    collector = nc.sync.nop(nofuse=True, hint="tile_tail_wait")
    wait_clock.add_sem_waits(collector.ins, ScopedClock({None: tick_clock.global_clock}))
    si = collector.ins.sync_info
    waits = list(si.on_wait) if si else []
    if si:
        collector.ins.sync_info = mybir.SyncInfo(on_wait=[], on_update=[])
    assert self.sems is not None
    name2sem = {s.name: s for s in self.sems.allocated().values()}
    for w in waits:
        nc.sync.wait_op(name2sem[w.ant_name], w.wait_value, _MODE_MAP.get(w.wait_mode, "sem-ge"))
    nc.all_engine_barrier(sem_only=True)
    popped = nc._tile_sem_poison_stack.pop()
    assert popped is self._sem_poison
    nc.clear_and_free_semaphores(list(self.sems.allocated().values()))
    nc.all_engine_barrier(sem_only=True)


tile.TileContext._drain_and_barrier = _patched_drain_and_barrier


def _fixup_sync_waits(nc):
    """This walrus build accepts at most 1 sync wait per compute/DMA
    instruction (EventSemaphore: 2). Tile's add_semaphores can emit more.
    Hoist excess waits onto EventSemaphore instructions inserted just
    before the over-budget instruction on the same engine."""
    for bb in nc.main_func.blocks:
        insts = bb.instructions
        out = []
        changed = False
        for ins in insts:
            si = ins.sync_info
            cap = 2 if type(ins).__name__ == "InstEventSemaphore" else 1
            if si is not None and len(si.on_wait) > cap:
                waits = list(si.on_wait)
                keep, excess = waits[-1:], waits[:-1]
                for i in range(0, len(excess), 2):
                    ev = mybir.InstEventSemaphore(
                        name=nc.get_next_instruction_name(),
                        ins=[], outs=[],
                        sync_info=mybir.SyncInfo(on_wait=excess[i:i + 2], on_update=[]),
                    )
                    ev.engine = ins.engine
                    out.append(ev)
                ins.sync_info = mybir.SyncInfo(on_wait=keep, on_update=list(si.on_update))
                changed = True
            out.append(ins)
        if changed:
            bb.instructions = out


def _emit_kernel(nc):
    qT = nc.dram_tensor("qT", [D, T], bf16, kind="ExternalInput")
    kT = nc.dram_tensor("kT", [D, T], bf16, kind="ExternalInput")
    vT = nc.dram_tensor("vT", [D, T], bf16, kind="ExternalInput")
    wq = nc.dram_tensor("wqT", [D, DH], bf16, kind="ExternalInput")
    wk = nc.dram_tensor("wkT", [D, DH], bf16, kind="ExternalInput")
    wv = nc.dram_tensor("wvT", [D, DH], bf16, kind="ExternalInput")
    wo = nc.dram_tensor("woT", [DH, D], bf16, kind="ExternalInput")
    bq = nc.dram_tensor("bq", [128, 4], f32, kind="ExternalInput")
    bk = nc.dram_tensor("bk", [128, 4], f32, kind="ExternalInput")
    bv = nc.dram_tensor("bv", [128, DH], f32, kind="ExternalInput")
    bo = nc.dram_tensor("bo", [128, 8], f32, kind="ExternalInput")
    tri = nc.dram_tensor("tri", [128, 128], bf16, kind="ExternalInput")
    # Each core outputs its head-group's full-D partial of its batch's
    # out^T (b_o/2 included); the host sums the two partials per batch.
    outT = nc.dram_tensor("outT", [D, T], bf16, kind="ExternalOutput")

    with tile.TileContext(nc, num_cores=N_CORES) as tc, ExitStack() as ctx:
        const = ctx.enter_context(tc.tile_pool(name="const", bufs=1))
        perm = ctx.enter_context(tc.tile_pool(name="perm", bufs=1))

        # Persistent on-chip tensors: [p, i, t] = full[i*128+p, t]
        QT = perm.tile([128, 4, T], bf16)
        KT = perm.tile([128, 4, T], bf16)
        Vg = perm.tile([128, 16, HPC * 65], bf16)   # V_aug: per k-tile, 8 heads x (64 vals + 1 one)
        xT = perm.tile([128, 4, T], bf16)

        bq_t = const.tile([128, 4], f32)
        bk_t = const.tile([128, 4], f32)
        bv_t = const.tile([128, DH], f32)
        bo_t = const.tile([128, 8], f32)
        tri_t = const.tile([128, 128], bf16)
        ones_t = const.tile([65, 64], f32r)
        nc.gpsimd.memset(ones_t[:].bitcast(f32), 1.0)
        nc.sync.dma_start(bq_t[:], bq[:])
        nc.sync.dma_start(bk_t[:], bk[:])
        nc.sync.dma_start(bv_t[:], bv[:])
        nc.sync.dma_start(bo_t[:], bo[:])
        nc.sync.dma_start(tri_t[:], tri[:])
        # ones column of V_aug, written once (columns 64 + 65*n, uniform stride)
        nc.gpsimd.memset(Vg[:].rearrange("p i (h j) -> p (i h) j", j=65)[:, :, 64:65], 1.0)

        # PE warmup: dependency-free matmuls that fill the initial input-DMA
        # wait so the HAM clock gate is released before the real work starts.
        with tc.tile_pool(name="warm", bufs=1) as warm, \
                tc.tile_pool(name="warm_psum", bufs=2, space="PSUM") as warm_psum:
            wrm = warm.tile([64, 512], f32r)
            nc.gpsimd.memset(wrm[:].bitcast(f32), 0.0)
            for _ in range(14):
                wp = warm_psum.tile([64, 512], f32)
                nc.tensor.matmul(wp[:], ones_t[0:64, :], wrm[:])

        # ---------------- fused projection + attention stream ----------------
        # The Q/K/V projections are woven INTO the attention pair-block loop:
        # each (hi, qc) iteration first projects exactly the Q/K e-block the
        # upcoming S-pair needs (Q tck=qc e=hi, K tck=qc e=hi), the V chunk
        # for token group qc is projected at hi==0, and the output projection
        # for group qc-1 lands at hi==1. This starts the ACT exp stream at
        # ~25us instead of after a ~105us projection prologue, and keeps
        # every engine's queue primed end-to-end.
        #
        # Heads are processed in PAIRS (2*hi, 2*hi+1): the even head's K/Q
        # rows live in SBUF partitions 0:64, the odd head's in 64:128, so
        # their S matmuls (contraction d_k=64) target disjoint PE row-groups
        # (tile_position auto-derives from base_partition) and run
        # CONCURRENTLY when issued interleaved -- 2x S throughput where a
        # lone d_k=64 matmul leaves half the array idle.
        with ExitStack() as ph:
            wpool = ph.enter_context(tc.tile_pool(name="wproj", bufs=3))
            wopool = ph.enter_context(tc.tile_pool(name="wopool", bufs=1))
            xpool = ph.enter_context(tc.tile_pool(name="xchunk", bufs=5))
            opool = ph.enter_context(tc.tile_pool(name="opool", bufs=3))
            ppool = ph.enter_context(tc.tile_pool(name="ppool", bufs=20))
            rpool = ph.enter_context(tc.tile_pool(name="rpool", bufs=4))
            s_psum = ph.enter_context(tc.tile_pool(name="s_psum", bufs=2, space="PSUM"))
            op_psum = ph.enter_context(tc.tile_pool(name="op_psum", bufs=2, space="PSUM"))
            pv_psum = ph.enter_context(tc.tile_pool(name="pv_psum", bufs=2, space="PSUM"))

            qsrc = qT.rearrange("(i p) t -> p i t", p=128)
            ksrc = kT.rearrange("(i p) t -> p i t", p=128)
            vsrc = vT.rearrange("(i p) t -> p i t", p=128)
            bv3 = bv_t[:].rearrange("p (h j) -> p h j", h=HPC)

            def stage_chunk(src, tck):
                xc = xpool.tile([128, 8, 512], bf16, tag="xchunk")
                for kt in range(8):
                    nc.sync.dma_start(xc[:, kt, :], src[:, kt, tck * 512:(tck + 1) * 512])
                return xc

            # DMA issue order follows first use: the first Q/K proj groups
            # need wq + the qc=0 Q chunk, then wk + K chunk; V and the
            # output-projection weights come later in the stream.
            wqt = wpool.tile([128, 8, DH], bf16, tag="wproj")
            wkt = wpool.tile([128, 8, DH], bf16, tag="wproj")
            wvt = wpool.tile([128, 8, DH], bf16, tag="wproj")
            chunks = {}
            for kt in range(8):
                nc.sync.dma_start(wqt[:, kt, :], wq.rearrange("(i p) n -> p i n", p=128)[:, kt, :])
            qxc0 = stage_chunk(qsrc, 0)
            for kt in range(8):
                nc.sync.dma_start(wkt[:, kt, :], wk.rearrange("(i p) n -> p i n", p=128)[:, kt, :])
            kxc0 = stage_chunk(ksrc, 0)
            for kt in range(8):
                nc.sync.dma_start(wvt[:, kt, :], wv.rearrange("(i p) n -> p i n", p=128)[:, kt, :])
            chunks[0] = (qxc0, kxc0, stage_chunk(vsrc, 0))
            wot = wopool.tile([128, 4, D], bf16)
            wosrc = wo.rearrange("(i p) n -> p i n", p=128)
            for kt in range(4):
                nc.sync.dma_start(wot[:, kt, :], wosrc[:, kt, :])

            def emit_qk_group(xc, wt_, dst, bias, tck, e):
                ps = op_psum.tile([128, 512], f32, tag="ops")
                for kt in range(8):
                    nc.tensor.matmul(
                        ps[:],
                        wt_[:, kt, e * 128:(e + 1) * 128],
                        xc[:, kt, :],
                        start=(kt == 0), stop=(kt == 7),
                    )
                nc.vector.tensor_scalar_add(
                    dst[:, e, tck * 512:(tck + 1) * 512], ps[:],
                    bias[:, e:e + 1],
                )

            def emit_v_chunk(tg, vxc):
                for tt in range(4):
                    ps = op_psum.tile([128, DH], f32, tag="ops")
                    for kt in range(8):
                        nc.tensor.matmul(
                            ps[:],
                            vxc[:, kt, tt * 128:(tt + 1) * 128],
                            wvt[:, kt, :],
                            start=(kt == 0), stop=(kt == 7),
                        )
                    ti = tg * 4 + tt
                    nc.vector.tensor_add(
                        Vg[:, ti, :].rearrange("p (h j) -> p h j", h=HPC)[:, :, 0:64],
                        ps[:].rearrange("p (h j) -> p h j", h=HPC),
                        bv3,
                    )

            def emit_outproj(tck):
                for e in range(8):
                    ps = op_psum.tile([128, 512], f32, tag="ops")
                    for kt in range(4):
                        nc.tensor.matmul(
                            ps[:],
                            wot[:, kt, e * 128:(e + 1) * 128],
                            xT[:, kt, tck * 512:(tck + 1) * 512],
                            start=(kt == 0), stop=(kt == 3),
                        )
                    ot = opool.tile([128, 512], bf16, tag="otile")
                    nc.vector.tensor_scalar_add(ot[:], ps[:], bo_t[:, e:e + 1])
                    nc.sync.dma_start(
                        outT[e * 128:(e + 1) * 128, tck * 512:(tck + 1) * 512],
                        ot[:],
                    )

            def emit_normalize(pv, rr, po, hi, q0):
                # Replicate 1/d across 64 partitions via PE outer product
                # (this walrus build lacks gpsimd partition_broadcast), then
                # DVE copy + mul (DVE reads only one PSUM operand per inst).
                rp = op_psum.tile([64, 512], f32, tag="ops")
                nc.tensor.matmul(rp[:], ones_t[64:65, :], rr[64:65, :])
                nc.vector.tensor_copy(xT[po:po + 64, hi, q0:q0 + 512], pv[0:64, :])
                nc.vector.tensor_mul(
                    xT[po:po + 64, hi, q0:q0 + 512],
                    xT[po:po + 64, hi, q0:q0 + 512],
                    rp[:],
                )

            def emit_pv(ptiles, h, qc):
                po = 64 * (h % 2)
                hi = h // 2
                q0 = qc * 512
                nkt = 4 * qc + 4
                pv = pv_psum.tile([65, 512], f32)
                for kt in range(nkt):
                    pt, off, vs, w = ptiles[kt]
                    nc.tensor.matmul(
                        pv[:, vs:512],
                        Vg[:, kt, 65 * h:65 * (h + 1)],
                        pt[:, off:off + w],
                        start=(kt == 0), stop=(kt == nkt - 1),
                    )
                rr = rpool.tile([65, 512], f32r, tag="rrow")
                with nc.allow_low_precision(reason="softmax denom recip in f32r"):
                    if qc < 2 or h % 2 == 1:
                        # ACT ln/exp reciprocal (~1.4us, shares the exp table
                        # set) where the DVE lane-serial reciprocal (~3.4us)
                        # would pace the block; split so neither engine
                        # saturates.
                        srl = rpool.tile([1, 512], f32, tag="srl")
                        nc.scalar.activation(srl[:], pv[64:65, :], LNF)
                        nc.scalar.activation(rr[64:65, :], srl[:], EXPF, scale=-1.0)
                    else:
                        nc.vector.reciprocal(rr[64:65, :], pv[64:65, :])
                return (pv, rr, po, hi, q0)

            def emit_spairs(hi, qc):
                """S + exp for the head pair (2*hi, 2*hi+1), matmuls issued
                alternating row-halves so the two heads' S tiles run
                concurrently in the PE array."""
                q0 = qc * 512
                ptA, ptB = [], []
                # fully-allowed tiles (kt < 4*qc): per-head [128,1024] spans
                # (2 k-tiles), one exp instruction per span
                for m in range(2 * qc):
                    spA = s_psum.tile([128, 1024], f32, tag="spair")
                    ptA2 = ppool.tile([128, 1024], bf16, tag="ppair", bufs=8)
                    spB = s_psum.tile([128, 1024], f32, tag="spair")
                    ptB2 = ppool.tile([128, 1024], bf16, tag="ppair", bufs=8)
                    for half in range(2):
                        kt = 2 * m + half
                        c0 = half * 512
                        nc.tensor.matmul(
                            spA[:, c0:c0 + 512],
                            KT[0:64, hi, kt * 128:(kt + 1) * 128],
                            QT[0:64, hi, q0:q0 + 512],
                        )
                        nc.tensor.matmul(
                            spB[:, c0:c0 + 512],
                            KT[64:128, hi, kt * 128:(kt + 1) * 128],
                            QT[64:128, hi, q0:q0 + 512],
                        )
                    nc.scalar.activation(ptA2[:], spA[:], EXPF, scale=0.125)
                    nc.scalar.activation(ptB2[:], spB[:], EXPF, scale=0.125)
                    ptA.append((ptA2, 0, 0, 512))
                    ptA.append((ptA2, 512, 0, 512))
                    ptB.append((ptB2, 0, 0, 512))
                    ptB.append((ptB2, 512, 0, 512))
                # diagonal-region tiles, width-trimmed pairs (512|384) and
                # (256|128) packed per head into one [128,1024] PSUM span
                for dp in range(2):
                    spA = s_psum.tile([128, 1024], f32, tag="spair")
                    ptA2 = ppool.tile([128, 1024], bf16, tag="ppair", bufs=8)
                    spB = s_psum.tile([128, 1024], f32, tag="spair")
                    ptB2 = ppool.tile([128, 1024], bf16, tag="ppair", bufs=8)
                    pair_off = 0
                    for half in range(2):
                        j = 2 * dp + half
                        kt = 4 * qc + j
                        vs = j * 128
                        w = 512 - vs
                        nc.tensor.matmul(
                            spA[:, pair_off:pair_off + w],
                            KT[0:64, hi, kt * 128:(kt + 1) * 128],
                            QT[0:64, hi, q0 + vs:q0 + 512],
                        )
                        nc.tensor.matmul(
                            spB[:, pair_off:pair_off + w],
                            KT[64:128, hi, kt * 128:(kt + 1) * 128],
                            QT[64:128, hi, q0 + vs:q0 + 512],
                        )
                        ptA.append((ptA2, pair_off, vs, w))
                        ptB.append((ptB2, pair_off, vs, w))
                        pair_off += w
                    nc.scalar.activation(
                        ptA2[:, 0:pair_off], spA[:, 0:pair_off],
                        EXPF, scale=0.125,
                    )
                    nc.scalar.activation(
                        ptB2[:, 0:pair_off], spB[:, 0:pair_off],
                        EXPF, scale=0.125,
                    )
                    for half in range(2):
                        off = ptA[-2 + half][1]
                        nc.gpsimd.tensor_mul(
                            ptA2[:, off:off + 128], ptA2[:, off:off + 128],
                            tri_t[:],
                        )
                        nc.gpsimd.tensor_mul(
                            ptB2[:, off:off + 128], ptB2[:, off:off + 128],
                            tri_t[:],
                        )
                return ptA, ptB

            # Two-deep software pipeline over the 16 (hi, qc) pair-blocks.
            # Q/K proj groups are emitted one BLOCK ahead of the S-pair that
            # consumes them, so the S matmuls never wait on the bias-add DVE
            # write of QT/KT emitted in the same iteration; chunk staging for
            # qc+1 fires at hi==2 (one block of DMA lead time).
            def emit_qk_for(qc_, hi_):
                qxc, kxc, _ = chunks[qc_]
                emit_qk_group(qxc, wqt, QT, bq_t, qc_, hi_)
                emit_qk_group(kxc, wkt, KT, bk_t, qc_, hi_)

            emit_qk_for(0, 0)
            pending_pv = None
            pending_nm = None
            pending_op = None
            for qc in range(4):
                for hi in range(4):
                    if hi == 2 and qc < 3:
                        chunks[qc + 1] = (
                            stage_chunk(qsrc, qc + 1),
                            stage_chunk(ksrc, qc + 1),
                            stage_chunk(vsrc, qc + 1),
                        )
                    if (qc, hi) != (3, 3):
                        nqc, nhi = (qc, hi + 1) if hi < 3 else (qc + 1, 0)
                        emit_qk_for(nqc, nhi)
                    if pending_pv is not None:
                        pA, pB, phi, pqc = pending_pv
                        nmA = emit_pv(pA, 2 * phi, pqc)
                        nmB = emit_pv(pB, 2 * phi + 1, pqc)
                        if pending_nm is not None:
                            for nm in pending_nm:
                                emit_normalize(*nm)
                        pending_nm = [nmA, nmB]
                        pending_pv = None
                    ptAB = emit_spairs(hi, qc)
                    if hi == 0:
                        emit_v_chunk(qc, chunks[qc][2])
                    if hi == 1 and pending_op is not None:
                        emit_outproj(pending_op)
                        pending_op = None
                    pending_pv = ptAB + (hi, qc)

                # group flush: finish every block of this qc so its xT columns
                # are final; the outproj emission is deferred into qc+1
                pA, pB, phi, pqc = pending_pv
                nmA = emit_pv(pA, 2 * phi, pqc)
                nmB = emit_pv(pB, 2 * phi + 1, pqc)
                pending_pv = None
                if pending_nm is not None:
                    for nm in pending_nm:
                        emit_normalize(*nm)
                emit_normalize(*nmA)
                emit_normalize(*nmB)
                pending_nm = None
                pending_op = qc
            emit_outproj(pending_op)

_NC_CACHE = None


def _build_nc():
    global _NC_CACHE
    if _NC_CACHE is None:
        nc = bass.Bass("TRN2", target_bir_lowering=False, debug=False, num_devices=N_CORES)
        _emit_kernel(nc)
        _fixup_sync_waits(nc)
        _NC_CACHE = nc
    return _NC_CACHE


def _host_mask_tiles(attention_mask, key_padding_mask):
    # The kernel exploits the causal structure; verify the runtime masks
    # actually match it (they do for this problem's setup_inputs()).
    am = np.asarray(attention_mask)[0]
    causal = np.triu(np.ones((T, T), np.int32), k=1)
    if not np.array_equal(am != 0, causal != 0):
        raise ValueError("kernel specialised for strict-upper-triangular causal mask")
    if np.asarray(key_padding_mask).any():
        raise ValueError("kernel specialised for all-attendable key_padding_mask")
    # 0/1 multiplicative mask for the 128x128 diagonal block of P^T[k, q]:
    # allowed iff q >= k.
    dk = np.arange(128)[:, None]
    dq = np.arange(128)[None, :]
    return (dq >= dk).astype(np.float32)


def _make_in_maps(inputs):
    import ml_dtypes
    bf = ml_dtypes.bfloat16
    query = np.asarray(inputs["query"], np.float32).astype(bf)
    key = np.asarray(inputs["key"], np.float32).astype(bf)
    value = np.asarray(inputs["value"], np.float32).astype(bf)
    W = {n: np.asarray(inputs[n], np.float32).astype(bf)
         for n in ("W_q", "W_k", "W_v", "W_o")}
    b = {n: np.asarray(inputs[n], np.float32) for n in ("b_q", "b_k", "b_v", "b_o")}
    msk = _host_mask_tiles(inputs["attention_mask"], inputs["key_padding_mask"])

    in_maps = []
    for c in range(N_CORES):
        bb, g = c // 2, c % 2
        hsel = slice(DH * g, DH * (g + 1))
        in_maps.append({
            "qT": np.ascontiguousarray(query[bb].T),
            "kT": np.ascontiguousarray(key[bb].T),
            "vT": np.ascontiguousarray(value[bb].T),
            "wqT": np.ascontiguousarray(W["W_q"].T[:, hsel]),
            "wkT": np.ascontiguousarray(W["W_k"].T[:, hsel]),
            "wvT": np.ascontiguousarray(W["W_v"].T[:, hsel]),
            "woT": np.ascontiguousarray(W["W_o"].T[hsel, :]),
            "bq": np.ascontiguousarray(b["b_q"][hsel].reshape(4, 128).T),
            "bk": np.ascontiguousarray(b["b_k"][hsel].reshape(4, 128).T),
            "bv": np.tile(b["b_v"][hsel][None, :], (128, 1)),
            "bo": np.ascontiguousarray((0.5 * b["b_o"]).reshape(8, 128).T),
            "tri": msk.astype(bf),
        })
    return in_maps


def assemble_output(results):
    """Sum the two per-head-group partial out^T per batch (host-side
    unshard: each core's outT is its head-group's full-D partial)."""
    out = np.empty((B, T, D), np.float32)
    for bb in range(B):
        p0 = np.asarray(results[2 * bb]["outT"], np.float32)
        p1 = np.asarray(results[2 * bb + 1]["outT"], np.float32)
        out[bb] = (p0 + p1).T
    return out


def kernel(**inputs):
    nc = _build_nc()
    in_maps = _make_in_maps(inputs)
    out = None
    for _attempt in range(3):
        res = run_bass_kernel_spmd(nc, in_maps, core_ids=list(range(N_CORES)))
        out = assemble_output(res.results)
        # rare transient device flake can corrupt an execution; retry
        if np.isfinite(out).all():
            break
    return out



# revision 17
# speedup vs baseline: 1.1748x; 1.0671x over previous
"""Multi-head attention (B=4, T=2048, D=1024, H=16, causal) on 8 trn2 NeuronCores.

Sharding: core c handles batch b = c//2 and head-group g = c%2 (8 heads,
512 model dims). Q/K/V projections are computed per-core for the core's
head slice (W_q/W_k/W_v column-sharded), attention runs fully on-core,
the output projection uses W_o row-sharded. Each core DMAs its full-D
partial out^T (its head-group's contribution, with b_o/2 folded in) and
the host sums the two partials per batch -- no on-device collective,
which removes the init barrier and the ~30us ReduceScatter tail.

Matmul operands are bf16 (PSUM accumulation stays fp32): halves HBM/SBUF
traffic and eases the PE power throttle that full-rate fp32 trips, at
rel-err ~4e-3 (tolerance 2e-2). Activations are kept transposed
([d, tokens]) on-chip so every matmul operand is naturally K-major:
    Q^T = Wq^T.T @ X^T           (per 128-d' tile, accumulated in PSUM)
    S^T[k,q] = (K^T slice).T @ Q^T slice      (contraction d_k = 64)
    P^T = exp(S^T / 8) with the strict-upper-triangle masked: fully-masked
        column ranges are simply skipped by the PV accumulation, the
        128x128 diagonal block is masked by a 0/1 multiply on GpSimd.
        Diagonal-region tiles are packed width-trimmed in pairs into one
        2-bank PSUM span so one exp instruction covers both (ACT insts
        are ~293ns fixed + 0.83ns/col, so fewer/wider is cheaper).
    [x^T | s] = V_aug.T @ P^T    (V augmented with a ones column -> row sums)
    x^T normalized by s via recip (ACT ln/exp or DVE) + PE outer-product
        replicate + one DVE mul straight from the two PSUM operands
    out^T partial = Wo^T.T @ x^T, + b_o/2 via DVE tensor_scalar_add
        (bias adds live on DVE, keeping ACT exp-only: ACT paces the
        attention phase), DMA straight to outT.
"""
import os
import numpy as np
from contextlib import ExitStack

import concourse.bass as bass
import concourse.tile as tile
import concourse.mybir as mybir
from concourse.bass_utils import run_bass_kernel_spmd
from bass_rust import ScopedClock

f32 = mybir.dt.float32
f32r = mybir.dt.float32r
bf16 = mybir.dt.bfloat16
EXPF = mybir.ActivationFunctionType.Exp
LNF = mybir.ActivationFunctionType.Ln
IDENT = mybir.ActivationFunctionType.Identity

B, T, D = 4, 2048, 1024
H, DK = 16, 64
N_CORES = 8
HPC = 8            # heads per core
DH = HPC * DK      # 512, model dims per core
NEG = -1.0e9

_MODE_MAP = {"sem-ge-imm": "sem-ge", "sem-eq-imm": "sem-eq", "sem-le-imm": "sem-le"}


def _patched_drain_and_barrier(self, tick_clock, wait_clock):
    # This walrus build rejects Drain/NoOp instructions that carry sync
    # waits ("Too many sync wait commands"), which the stock Tile tail
    # emits. Put the tail waits on individual EventSemaphore instructions
    # and use sem-only barriers instead of the drain butterfly.
    nc = self.nc
    collector = nc.sync.nop(nofuse=True, hint="tile_tail_wait")
    wait_clock.add_sem_waits(collector.ins, ScopedClock({None: tick_clock.global_clock}))
    si = collector.ins.sync_info
    waits = list(si.on_wait) if si else []
    if si:
        collector.ins.sync_info = mybir.SyncInfo(on_wait=[], on_update=[])
    assert self.sems is not None
    name2sem = {s.name: s for s in self.sems.allocated().values()}
    for w in waits:
        nc.sync.wait_op(name2sem[w.ant_name], w.wait_value, _MODE_MAP.get(w.wait_mode, "sem-ge"))
    nc.all_engine_barrier(sem_only=True)
    popped = nc._tile_sem_poison_stack.pop()
    assert popped is self._sem_poison
    nc.clear_and_free_semaphores(list(self.sems.allocated().values()))
    nc.all_engine_barrier(sem_only=True)


tile.TileContext._drain_and_barrier = _patched_drain_and_barrier


def _fixup_sync_waits(nc):
    """This walrus build accepts at most 1 sync wait per compute/DMA
    instruction (EventSemaphore: 2). Tile's add_semaphores can emit more.
    Hoist excess waits onto EventSemaphore instructions inserted just
    before the over-budget instruction on the same engine."""
    for bb in nc.main_func.blocks:
        insts = bb.instructions
        out = []
        changed = False
        for ins in insts:
            si = ins.sync_info
            cap = 2 if type(ins).__name__ == "InstEventSemaphore" else 1
            if si is not None and len(si.on_wait) > cap:
                waits = list(si.on_wait)
                keep, excess = waits[-1:], waits[:-1]
                for i in range(0, len(excess), 2):
                    ev = mybir.InstEventSemaphore(
                        name=nc.get_next_instruction_name(),
                        ins=[], outs=[],
                        sync_info=mybir.SyncInfo(on_wait=excess[i:i + 2], on_update=[]),
                    )
                    ev.engine = ins.engine
                    out.append(ev)
                ins.sync_info = mybir.SyncInfo(on_wait=keep, on_update=list(si.on_update))
                changed = True
            out.append(ins)
        if changed:
            bb.instructions = out


def _emit_kernel(nc):
    qT = nc.dram_tensor("qT", [D, T], bf16, kind="ExternalInput")
    kT = nc.dram_tensor("kT", [D, T], bf16, kind="ExternalInput")
    vT = nc.dram_tensor("vT", [D, T], bf16, kind="ExternalInput")
    wq = nc.dram_tensor("wqT", [D, DH], bf16, kind="ExternalInput")
    wk = nc.dram_tensor("wkT", [D, DH], bf16, kind="ExternalInput")
    wv = nc.dram_tensor("wvT", [D, DH], bf16, kind="ExternalInput")
    wo = nc.dram_tensor("woT", [DH, D], bf16, kind="ExternalInput")
    bq = nc.dram_tensor("bq", [128, 4], f32, kind="ExternalInput")
    bk = nc.dram_tensor("bk", [128, 4], f32, kind="ExternalInput")
    bv = nc.dram_tensor("bv", [128, DH], f32, kind="ExternalInput")
    bo = nc.dram_tensor("bo", [128, 8], f32, kind="ExternalInput")
    tri = nc.dram_tensor("tri", [128, 128], bf16, kind="ExternalInput")
    # Each core outputs its head-group's full-D partial of its batch's
    # out^T (b_o/2 included); the host sums the two partials per batch.
    outT = nc.dram_tensor("outT", [D, T], bf16, kind="ExternalOutput")

    with tile.TileContext(nc, num_cores=N_CORES) as tc, ExitStack() as ctx:
        const = ctx.enter_context(tc.tile_pool(name="const", bufs=1))
        perm = ctx.enter_context(tc.tile_pool(name="perm", bufs=1))

        # Persistent on-chip tensors: [p, i, t] = full[i*128+p, t]
        QT = perm.tile([128, 4, T], bf16)
        KT = perm.tile([128, 4, T], bf16)
        Vg = perm.tile([128, 16, HPC * 65], bf16)   # V_aug: per k-tile, 8 heads x (64 vals + 1 one)
        xT = perm.tile([128, 4, T], bf16)

        bq_t = const.tile([128, 4], f32)
        bk_t = const.tile([128, 4], f32)
        bv_t = const.tile([128, DH], f32)
        bo_t = const.tile([128, 8], f32)
        tri_t = const.tile([128, 128], bf16)
        ones_t = const.tile([65, 64], f32r)
        nc.gpsimd.memset(ones_t[:].bitcast(f32), 1.0)
        nc.sync.dma_start(bq_t[:], bq[:])
        nc.sync.dma_start(bk_t[:], bk[:])
        nc.sync.dma_start(bv_t[:], bv[:])
        nc.sync.dma_start(bo_t[:], bo[:])
        nc.sync.dma_start(tri_t[:], tri[:])
        # ones column of V_aug, written once (columns 64 + 65*n, uniform stride)
        nc.gpsimd.memset(Vg[:].rearrange("p i (h j) -> p (i h) j", j=65)[:, :, 64:65], 1.0)

        # PE warmup: dependency-free matmuls that fill the initial input-DMA
        # wait so the HAM clock gate is released before the real work starts.
        with tc.tile_pool(name="warm", bufs=1) as warm, \
                tc.tile_pool(name="warm_psum", bufs=2, space="PSUM") as warm_psum:
            wrm = warm.tile([64, 512], f32r)
            nc.gpsimd.memset(wrm[:].bitcast(f32), 0.0)
            for _ in range(14):
                wp = warm_psum.tile([64, 512], f32)
                nc.tensor.matmul(wp[:], ones_t[0:64, :], wrm[:])

        # ---------------- Q / K projections ----------------
        with ExitStack() as ph:
            wpool = ph.enter_context(tc.tile_pool(name="wproj", bufs=2))
            xpool = ph.enter_context(tc.tile_pool(name="xchunk", bufs=3))
            qk_psum = ph.enter_context(tc.tile_pool(name="qk_psum", bufs=4, space="PSUM"))
            v_psum = ph.enter_context(tc.tile_pool(name="v_psum", bufs=4, space="PSUM"))

            for name, wdram, xdram, dst, bias in (
                ("q", wq, qT, QT, bq_t),
                ("k", wk, kT, KT, bk_t),
            ):
                wt = wpool.tile([128, 8, DH], bf16, tag="wproj")
                wsrc = wdram.rearrange("(i p) n -> p i n", p=128)
                for kt in range(8):
                    nc.sync.dma_start(wt[:, kt, :], wsrc[:, kt, :])
                xsrc = xdram.rearrange("(i p) t -> p i t", p=128)
                for tck in range(4):
                    xc = xpool.tile([128, 8, 512], bf16, tag="xchunk")
                    for kt in range(8):
                        nc.sync.dma_start(xc[:, kt, :], xsrc[:, kt, tck * 512:(tck + 1) * 512])
                    for e in range(4):
                        ps = qk_psum.tile([128, 512], f32)
                        for kt in range(8):
                            nc.tensor.matmul(
                                ps[:],
                                wt[:, kt, e * 128:(e + 1) * 128],
                                xc[:, kt, :],
                                start=(kt == 0), stop=(kt == 7),
                            )
                        nc.vector.tensor_scalar_add(
                            dst[:, e, tck * 512:(tck + 1) * 512], ps[:],
                            bias[:, e:e + 1],
                        )

            # ---------------- V projection (natural layout, into V_aug) ----------------
            wvt = wpool.tile([128, 8, DH], bf16, tag="wproj")
            nc.sync.dma_start(wvt[:], wv.rearrange("(i p) n -> p i n", p=128))
            vsrc = vT.rearrange("(i p) t -> p i t", p=128)
            bv3 = bv_t[:].rearrange("p (h j) -> p h j", h=HPC)
            for tg in range(4):
                xc = xpool.tile([128, 8, 512], bf16, tag="xchunk")
                for kt in range(8):
                    nc.sync.dma_start(xc[:, kt, :], vsrc[:, kt, tg * 512:(tg + 1) * 512])
                for tt in range(4):
                    ps = v_psum.tile([128, DH], f32)
                    for kt in range(8):
                        nc.tensor.matmul(
                            ps[:],
                            xc[:, kt, tt * 128:(tt + 1) * 128],
                            wvt[:, kt, :],
                            start=(kt == 0), stop=(kt == 7),
                        )
                    ti = tg * 4 + tt
                    nc.vector.tensor_add(
                        Vg[:, ti, :].rearrange("p (h j) -> p h j", h=HPC)[:, :, 0:64],
                        ps[:].rearrange("p (h j) -> p h j", h=HPC),
                        bv3,
                    )

        # ---------------- attention + interleaved output projection ----------------
        # Heads are processed in PAIRS (2*hi, 2*hi+1): the even head's K/Q
        # rows live in SBUF partitions 0:64, the odd head's in 64:128, so
        # their S matmuls (contraction d_k=64) target disjoint PE row-groups
        # (tile_position auto-derives from base_partition) and run
        # CONCURRENTLY when issued interleaved -- 2x S throughput where a
        # lone d_k=64 matmul leaves half the array idle.
        with ExitStack() as ph:
            wopool = ph.enter_context(tc.tile_pool(name="wopool", bufs=1))
            opool = ph.enter_context(tc.tile_pool(name="opool", bufs=3))
            ppool = ph.enter_context(tc.tile_pool(name="ppool", bufs=20))
            rpool = ph.enter_context(tc.tile_pool(name="rpool", bufs=4))
            s_psum = ph.enter_context(tc.tile_pool(name="s_psum", bufs=2, space="PSUM"))
            op_psum = ph.enter_context(tc.tile_pool(name="op_psum", bufs=2, space="PSUM"))
            pv_psum = ph.enter_context(tc.tile_pool(name="pv_psum", bufs=2, space="PSUM"))

            wot = wopool.tile([128, 4, D], bf16)
            wosrc = wo.rearrange("(i p) n -> p i n", p=128)
            for kt in range(4):
                nc.sync.dma_start(wot[:, kt, :], wosrc[:, kt, :])

            def emit_outproj(tck):
                for e in range(8):
                    ps = op_psum.tile([128, 512], f32, tag="ops")
                    for kt in range(4):
                        nc.tensor.matmul(
                            ps[:],
                            wot[:, kt, e * 128:(e + 1) * 128],
                            xT[:, kt, tck * 512:(tck + 1) * 512],
                            start=(kt == 0), stop=(kt == 3),
                        )
                    ot = opool.tile([128, 512], bf16, tag="otile")
                    nc.vector.tensor_scalar_add(ot[:], ps[:], bo_t[:, e:e + 1])
                    nc.sync.dma_start(
                        outT[e * 128:(e + 1) * 128, tck * 512:(tck + 1) * 512],
                        ot[:],
                    )

            pending_pv = None    # (ptA, ptB, hi, qc): S/exp emitted, PVs pending
            pending_nm = None    # list of (pv, rr, po, hi, q0): normalize pending

            def emit_normalize(pv, rr, po, hi, q0):
                # Replicate 1/d across 64 partitions via PE outer product
                # (this walrus build lacks gpsimd partition_broadcast), then
                # DVE copy + mul (DVE reads only one PSUM operand per inst).
                rp = op_psum.tile([64, 512], f32, tag="ops")
                nc.tensor.matmul(rp[:], ones_t[64:65, :], rr[64:65, :])
                nc.vector.tensor_copy(xT[po:po + 64, hi, q0:q0 + 512], pv[0:64, :])
                nc.vector.tensor_mul(
                    xT[po:po + 64, hi, q0:q0 + 512],
                    xT[po:po + 64, hi, q0:q0 + 512],
                    rp[:],
                )

            def emit_pv(ptiles, h, qc):
                po = 64 * (h % 2)
                hi = h // 2
                q0 = qc * 512
                nkt = 4 * qc + 4
                pv = pv_psum.tile([65, 512], f32)
                for kt in range(nkt):
                    pt, off, vs, w = ptiles[kt]
                    nc.tensor.matmul(
                        pv[:, vs:512],
                        Vg[:, kt, 65 * h:65 * (h + 1)],
                        pt[:, off:off + w],
                        start=(kt == 0), stop=(kt == nkt - 1),
                    )
                rr = rpool.tile([65, 512], f32r, tag="rrow")
                with nc.allow_low_precision(reason="softmax denom recip in f32r"):
                    if qc < 2 or h % 2 == 1:
                        # ACT ln/exp reciprocal (~1.4us, shares the exp table
                        # set) where the DVE lane-serial reciprocal (~3.4us)
                        # would pace the block; split so neither engine
                        # saturates.
                        srl = rpool.tile([1, 512], f32, tag="srl")
                        nc.scalar.activation(srl[:], pv[64:65, :], LNF)
                        nc.scalar.activation(rr[64:65, :], srl[:], EXPF, scale=-1.0)
                    else:
                        nc.vector.reciprocal(rr[64:65, :], pv[64:65, :])
                return (pv, rr, po, hi, q0)

            def emit_spairs(hi, qc):
                """S + exp for the head pair (2*hi, 2*hi+1), matmuls issued
                alternating row-halves so the two heads' S tiles run
                concurrently in the PE array."""
                q0 = qc * 512
                ptA, ptB = [], []
                # fully-allowed tiles (kt < 4*qc): per-head [128,1024] spans
                # (2 k-tiles), one exp instruction per span
                for m in range(2 * qc):
                    spA = s_psum.tile([128, 1024], f32, tag="spair")
                    ptA2 = ppool.tile([128, 1024], bf16, tag="ppair", bufs=10)
                    spB = s_psum.tile([128, 1024], f32, tag="spair")
                    ptB2 = ppool.tile([128, 1024], bf16, tag="ppair", bufs=10)
                    for half in range(2):
                        kt = 2 * m + half
                        c0 = half * 512
                        nc.tensor.matmul(
                            spA[:, c0:c0 + 512],
                            KT[0:64, hi, kt * 128:(kt + 1) * 128],
                            QT[0:64, hi, q0:q0 + 512],
                        )
                        nc.tensor.matmul(
                            spB[:, c0:c0 + 512],
                            KT[64:128, hi, kt * 128:(kt + 1) * 128],
                            QT[64:128, hi, q0:q0 + 512],
                        )
                    nc.scalar.activation(ptA2[:], spA[:], EXPF, scale=0.125)
                    nc.scalar.activation(ptB2[:], spB[:], EXPF, scale=0.125)
                    ptA.append((ptA2, 0, 0, 512))
                    ptA.append((ptA2, 512, 0, 512))
                    ptB.append((ptB2, 0, 0, 512))
                    ptB.append((ptB2, 512, 0, 512))
                # diagonal-region tiles, width-trimmed pairs (512|384) and
                # (256|128) packed per head into one [128,1024] PSUM span
                for dp in range(2):
                    spA = s_psum.tile([128, 1024], f32, tag="spair")
                    ptA2 = ppool.tile([128, 1024], bf16, tag="ppair", bufs=10)
                    spB = s_psum.tile([128, 1024], f32, tag="spair")
                    ptB2 = ppool.tile([128, 1024], bf16, tag="ppair", bufs=10)
                    pair_off = 0
                    for half in range(2):
                        j = 2 * dp + half
                        kt = 4 * qc + j
                        vs = j * 128
                        w = 512 - vs
                        nc.tensor.matmul(
                            spA[:, pair_off:pair_off + w],
                            KT[0:64, hi, kt * 128:(kt + 1) * 128],
                            QT[0:64, hi, q0 + vs:q0 + 512],
                        )
                        nc.tensor.matmul(
                            spB[:, pair_off:pair_off + w],
                            KT[64:128, hi, kt * 128:(kt + 1) * 128],
                            QT[64:128, hi, q0 + vs:q0 + 512],
                        )
                        ptA.append((ptA2, pair_off, vs, w))
                        ptB.append((ptB2, pair_off, vs, w))
                        pair_off += w
                    nc.scalar.activation(
                        ptA2[:, 0:pair_off], spA[:, 0:pair_off],
                        EXPF, scale=0.125,
                    )
                    nc.scalar.activation(
                        ptB2[:, 0:pair_off], spB[:, 0:pair_off],
                        EXPF, scale=0.125,
                    )
                    for half in range(2):
                        off = ptA[-2 + half][1]
                        nc.gpsimd.tensor_mul(
                            ptA2[:, off:off + 128], ptA2[:, off:off + 128],
                            tri_t[:],
                        )
                        nc.gpsimd.tensor_mul(
                            ptB2[:, off:off + 128], ptB2[:, off:off + 128],
                            tri_t[:],
                        )
                return ptA, ptB

            # Two-deep software pipeline over the 16 (hi, qc) pair-blocks:
            # [S(i+1)] [PV pair(i)] [norms(i-1)], so the PE never waits on
            # the exps (ACT) of the block it just produced. The output
            # projection for group qc is delayed until after the first
            # S-burst of group qc+1 so ACT has exp work while the PE runs
            # the outproj matmuls.
            pending_op = None
            for qc in range(4):
                for hi in range(4):
                    ptA, ptB = emit_spairs(hi, qc)
                    if pending_op is not None and hi == 1:
                        emit_outproj(pending_op)
                        pending_op = None
                    if pending_pv is not None:
                        pA, pB, phi, pqc = pending_pv
                        nmA = emit_pv(pA, 2 * phi, pqc)
                        nmB = emit_pv(pB, 2 * phi + 1, pqc)
                        if pending_nm is not None:
                            for nm in pending_nm:
                                emit_normalize(*nm)
                        pending_nm = [nmA, nmB]
                    pending_pv = (ptA, ptB, hi, qc)

                # group flush: finish every block of this qc so its xT columns
                # are final; the outproj emission is deferred into qc+1
                pA, pB, phi, pqc = pending_pv
                nmA = emit_pv(pA, 2 * phi, pqc)
                nmB = emit_pv(pB, 2 * phi + 1, pqc)
                pending_pv = None
                if pending_nm is not None:
                    for nm in pending_nm:
                        emit_normalize(*nm)
                emit_normalize(*nmA)
                emit_normalize(*nmB)
                pending_nm = None
                pending_op = qc
            emit_outproj(pending_op)


_NC_CACHE = None


def _build_nc():
    global _NC_CACHE
    if _NC_CACHE is None:
        nc = bass.Bass("TRN2", target_bir_lowering=False, debug=False, num_devices=N_CORES)
        _emit_kernel(nc)
        _fixup_sync_waits(nc)
        _NC_CACHE = nc
    return _NC_CACHE


def _host_mask_tiles(attention_mask, key_padding_mask):
    # The kernel exploits the causal structure; verify the runtime masks
    # actually match it (they do for this problem's setup_inputs()).
    am = np.asarray(attention_mask)[0]
    causal = np.triu(np.ones((T, T), np.int32), k=1)
    if not np.array_equal(am != 0, causal != 0):
        raise ValueError("kernel specialised for strict-upper-triangular causal mask")
    if np.asarray(key_padding_mask).any():
        raise ValueError("kernel specialised for all-attendable key_padding_mask")
    # 0/1 multiplicative mask for the 128x128 diagonal block of P^T[k, q]:
    # allowed iff q >= k.
    dk = np.arange(128)[:, None]
    dq = np.arange(128)[None, :]
    return (dq >= dk).astype(np.float32)


def _make_in_maps(inputs):
    import ml_dtypes
    bf = ml_dtypes.bfloat16
    query = np.asarray(inputs["query"], np.float32).astype(bf)
    key = np.asarray(inputs["key"], np.float32).astype(bf)
    value = np.asarray(inputs["value"], np.float32).astype(bf)
    W = {n: np.asarray(inputs[n], np.float32).astype(bf)
         for n in ("W_q", "W_k", "W_v", "W_o")}
    b = {n: np.asarray(inputs[n], np.float32) for n in ("b_q", "b_k", "b_v", "b_o")}
    msk = _host_mask_tiles(inputs["attention_mask"], inputs["key_padding_mask"])

    in_maps = []
    for c in range(N_CORES):
        bb, g = c // 2, c % 2
        hsel = slice(DH * g, DH * (g + 1))
        in_maps.append({
            "qT": np.ascontiguousarray(query[bb].T),
            "kT": np.ascontiguousarray(key[bb].T),
            "vT": np.ascontiguousarray(value[bb].T),
            "wqT": np.ascontiguousarray(W["W_q"].T[:, hsel]),
            "wkT": np.ascontiguousarray(W["W_k"].T[:, hsel]),
            "wvT": np.ascontiguousarray(W["W_v"].T[:, hsel]),
            "woT": np.ascontiguousarray(W["W_o"].T[hsel, :]),
            "bq": np.ascontiguousarray(b["b_q"][hsel].reshape(4, 128).T),
            "bk": np.ascontiguousarray(b["b_k"][hsel].reshape(4, 128).T),
            "bv": np.tile(b["b_v"][hsel][None, :], (128, 1)),
            "bo": np.ascontiguousarray((0.5 * b["b_o"]).reshape(8, 128).T),
            "tri": msk.astype(bf),
        })
    return in_maps


def assemble_output(results):
    """Sum the two per-head-group partial out^T per batch (host-side
    unshard: each core's outT is its head-group's full-D partial)."""
    out = np.empty((B, T, D), np.float32)
    for bb in range(B):
        p0 = np.asarray(results[2 * bb]["outT"], np.float32)
        p1 = np.asarray(results[2 * bb + 1]["outT"], np.float32)
        out[bb] = (p0 + p1).T
    return out


def kernel(**inputs):
    nc = _build_nc()
    in_maps = _make_in_maps(inputs)
    out = None
    for _attempt in range(3):
        res = run_bass_kernel_spmd(nc, in_maps, core_ids=list(range(N_CORES)))
        out = assemble_output(res.results)
        # rare transient device flake can corrupt an execution; retry
        if np.isfinite(out).all():
            break
    return out



# revision 22
# speedup vs baseline: 1.1849x; 1.0086x over previous
"""Multi-head attention (B=4, T=2048, D=1024, H=16, causal) on 8 trn2 NeuronCores.

Sharding: core c handles batch b = c//2 and head-group g = c%2 (8 heads,
512 model dims). Q/K/V projections are computed per-core for the core's
head slice (W_q/W_k/W_v column-sharded), attention runs fully on-core,
the output projection uses W_o row-sharded. Each core DMAs its full-D
partial out^T (its head-group's contribution, with b_o/2 folded in) and
the host sums the two partials per batch -- no on-device collective,
which removes the init barrier and the ~30us ReduceScatter tail.

Matmul operands are bf16 (PSUM accumulation stays fp32): halves HBM/SBUF
traffic and eases the PE power throttle that full-rate fp32 trips, at
rel-err ~4e-3 (tolerance 2e-2). Activations are kept transposed
([d, tokens]) on-chip so every matmul operand is naturally K-major:
    Q^T = Wq^T.T @ X^T           (per 128-d' tile, accumulated in PSUM)
    S^T[k,q] = (K^T slice).T @ Q^T slice      (contraction d_k = 64)
    P^T = exp(S^T / 8) with the strict-upper-triangle masked: fully-masked
        column ranges are simply skipped by the PV accumulation, the
        128x128 diagonal block is masked by a 0/1 multiply on GpSimd.
        Diagonal-region tiles are packed width-trimmed in pairs into one
        2-bank PSUM span so one exp instruction covers both (ACT insts
        are ~293ns fixed + 0.83ns/col, so fewer/wider is cheaper).
    [x^T | s] = V_aug.T @ P^T    (V augmented with a ones column -> row sums)
    x^T normalized by s via recip (ACT ln/exp or DVE) + PE outer-product
        replicate + one DVE mul straight from the two PSUM operands
    out^T partial = Wo^T.T @ x^T, + b_o/2 via DVE tensor_scalar_add
        (bias adds live on DVE, keeping ACT exp-only: ACT paces the
        attention phase), DMA straight to outT.
"""
import os
import numpy as np
from contextlib import ExitStack

import concourse.bass as bass
import concourse.tile as tile
import concourse.mybir as mybir
from concourse.bass_utils import run_bass_kernel_spmd
from bass_rust import ScopedClock

f32 = mybir.dt.float32
f32r = mybir.dt.float32r
bf16 = mybir.dt.bfloat16
EXPF = mybir.ActivationFunctionType.Exp
LNF = mybir.ActivationFunctionType.Ln
IDENT = mybir.ActivationFunctionType.Identity

B, T, D = 4, 2048, 1024
H, DK = 16, 64
N_CORES = 8
HPC = 8            # heads per core
DH = HPC * DK      # 512, model dims per core
NEG = -1.0e9

_MODE_MAP = {"sem-ge-imm": "sem-ge", "sem-eq-imm": "sem-eq", "sem-le-imm": "sem-le"}


def _patched_drain_and_barrier(self, tick_clock, wait_clock):
    # This walrus build rejects Drain/NoOp instructions that carry sync
    # waits ("Too many sync wait commands"), which the stock Tile tail
    # emits. Put the tail waits on individual EventSemaphore instructions
    # and use sem-only barriers instead of the drain butterfly.
    nc = self.nc
    collector = nc.sync.nop(nofuse=True, hint="tile_tail_wait")
    wait_clock.add_sem_waits(collector.ins, ScopedClock({None: tick_clock.global_clock}))
    si = collector.ins.sync_info
    waits = list(si.on_wait) if si else []
    if si:
        collector.ins.sync_info = mybir.SyncInfo(on_wait=[], on_update=[])
    assert self.sems is not None
    name2sem = {s.name: s for s in self.sems.allocated().values()}
    for w in waits:
        nc.sync.wait_op(name2sem[w.ant_name], w.wait_value, _MODE_MAP.get(w.wait_mode, "sem-ge"))
    nc.all_engine_barrier(sem_only=True)
    popped = nc._tile_sem_poison_stack.pop()
    assert popped is self._sem_poison
    nc.clear_and_free_semaphores(list(self.sems.allocated().values()))
    nc.all_engine_barrier(sem_only=True)


tile.TileContext._drain_and_barrier = _patched_drain_and_barrier


def _fixup_sync_waits(nc):
    """This walrus build accepts at most 1 sync wait per compute/DMA
    instruction (EventSemaphore: 2). Tile's add_semaphores can emit more.
    Hoist excess waits onto EventSemaphore instructions inserted just
    before the over-budget instruction on the same engine."""
    for bb in nc.main_func.blocks:
        insts = bb.instructions
        out = []
        changed = False
        for ins in insts:
            si = ins.sync_info
            cap = 2 if type(ins).__name__ == "InstEventSemaphore" else 1
            if si is not None and len(si.on_wait) > cap:
                waits = list(si.on_wait)
                keep, excess = waits[-1:], waits[:-1]
                for i in range(0, len(excess), 2):
                    ev = mybir.InstEventSemaphore(
                        name=nc.get_next_instruction_name(),
                        ins=[], outs=[],
                        sync_info=mybir.SyncInfo(on_wait=excess[i:i + 2], on_update=[]),
                    )
                    ev.engine = ins.engine
                    out.append(ev)
                ins.sync_info = mybir.SyncInfo(on_wait=keep, on_update=list(si.on_update))
                changed = True
            out.append(ins)
        if changed:
            bb.instructions = out


def _emit_kernel(nc):
    qT = nc.dram_tensor("qT", [D, T], bf16, kind="ExternalInput")
    kT = nc.dram_tensor("kT", [D, T], bf16, kind="ExternalInput")
    vT = nc.dram_tensor("vT", [D, T], bf16, kind="ExternalInput")
    wq = nc.dram_tensor("wqT", [D, DH], bf16, kind="ExternalInput")
    wk = nc.dram_tensor("wkT", [D, DH], bf16, kind="ExternalInput")
    wv = nc.dram_tensor("wvT", [D, DH], bf16, kind="ExternalInput")
    wo = nc.dram_tensor("woT", [DH, D], bf16, kind="ExternalInput")
    bq = nc.dram_tensor("bq", [128, 4], f32, kind="ExternalInput")
    bk = nc.dram_tensor("bk", [128, 4], f32, kind="ExternalInput")
    bv = nc.dram_tensor("bv", [128, DH], f32, kind="ExternalInput")
    bo = nc.dram_tensor("bo", [128, 8], f32, kind="ExternalInput")
    tri = nc.dram_tensor("tri", [128, 128], bf16, kind="ExternalInput")
    # Each core outputs its head-group's full-D partial of its batch's
    # out^T (b_o/2 included); the host sums the two partials per batch.
    outT = nc.dram_tensor("outT", [D, T], bf16, kind="ExternalOutput")

    with tile.TileContext(nc, num_cores=N_CORES) as tc, ExitStack() as ctx:
        const = ctx.enter_context(tc.tile_pool(name="const", bufs=1))
        perm = ctx.enter_context(tc.tile_pool(name="perm", bufs=1))

        # Persistent on-chip tensors: [p, i, t] = full[i*128+p, t]
        QT = perm.tile([128, 4, T], bf16)
        KT = perm.tile([128, 4, T], bf16)
        Vg = perm.tile([128, 16, HPC * 65], bf16)   # V_aug: per k-tile, 8 heads x (64 vals + 1 one)
        xT = perm.tile([128, 4, T], bf16)

        bq_t = const.tile([128, 4], f32)
        bk_t = const.tile([128, 4], f32)
        bv_t = const.tile([128, DH], f32)
        bo_t = const.tile([128, 8], f32)
        tri_t = const.tile([128, 128], bf16)
        ones_t = const.tile([65, 64], f32r)
        nc.gpsimd.memset(ones_t[:].bitcast(f32), 1.0)
        nc.sync.dma_start(bq_t[:], bq[:])
        nc.sync.dma_start(bk_t[:], bk[:])
        nc.sync.dma_start(bv_t[:], bv[:])
        nc.sync.dma_start(bo_t[:], bo[:])
        nc.sync.dma_start(tri_t[:], tri[:])
        # ones column of V_aug, written once (columns 64 + 65*n, uniform stride)
        nc.gpsimd.memset(Vg[:].rearrange("p i (h j) -> p (i h) j", j=65)[:, :, 64:65], 1.0)

        # PE warmup: dependency-free matmuls that fill the initial input-DMA
        # wait so the HAM clock gate is released before the real work starts.
        with tc.tile_pool(name="warm", bufs=1) as warm, \
                tc.tile_pool(name="warm_psum", bufs=2, space="PSUM") as warm_psum:
            wrm = warm.tile([64, 512], f32r)
            nc.gpsimd.memset(wrm[:].bitcast(f32), 0.0)
            for _ in range(14):
                wp = warm_psum.tile([64, 512], f32)
                nc.tensor.matmul(wp[:], ones_t[0:64, :], wrm[:])

        # ---------------- Q / K projections ----------------
        with ExitStack() as ph:
            wpool = ph.enter_context(tc.tile_pool(name="wproj", bufs=2))
            xpool = ph.enter_context(tc.tile_pool(name="xchunk", bufs=3))
            qk_psum = ph.enter_context(tc.tile_pool(name="qk_psum", bufs=4, space="PSUM"))
            v_psum = ph.enter_context(tc.tile_pool(name="v_psum", bufs=4, space="PSUM"))

            for name, wdram, xdram, dst, bias in (
                ("q", wq, qT, QT, bq_t),
                ("k", wk, kT, KT, bk_t),
            ):
                wt = wpool.tile([128, 8, DH], bf16, tag="wproj")
                wsrc = wdram.rearrange("(i p) n -> p i n", p=128)
                for kt in range(8):
                    nc.sync.dma_start(wt[:, kt, :], wsrc[:, kt, :])
                xsrc = xdram.rearrange("(i p) t -> p i t", p=128)
                for tck in range(4):
                    xc = xpool.tile([128, 8, 512], bf16, tag="xchunk")
                    for kt in range(8):
                        nc.sync.dma_start(xc[:, kt, :], xsrc[:, kt, tck * 512:(tck + 1) * 512])
                    for e in range(4):
                        ps = qk_psum.tile([128, 512], f32)
                        for kt in range(8):
                            nc.tensor.matmul(
                                ps[:],
                                wt[:, kt, e * 128:(e + 1) * 128],
                                xc[:, kt, :],
                                start=(kt == 0), stop=(kt == 7),
                            )
                        nc.vector.tensor_scalar_add(
                            dst[:, e, tck * 512:(tck + 1) * 512], ps[:],
                            bias[:, e:e + 1],
                        )

            # ---------------- V projection (natural layout, into V_aug) ----------------
            wvt = wpool.tile([128, 8, DH], bf16, tag="wproj")
            nc.sync.dma_start(wvt[:], wv.rearrange("(i p) n -> p i n", p=128))
            vsrc = vT.rearrange("(i p) t -> p i t", p=128)
            bv3 = bv_t[:].rearrange("p (h j) -> p h j", h=HPC)
            for tg in range(4):
                xc = xpool.tile([128, 8, 512], bf16, tag="xchunk")
                for kt in range(8):
                    nc.sync.dma_start(xc[:, kt, :], vsrc[:, kt, tg * 512:(tg + 1) * 512])
                for tt in range(4):
                    ps = v_psum.tile([128, DH], f32)
                    for kt in range(8):
                        nc.tensor.matmul(
                            ps[:],
                            xc[:, kt, tt * 128:(tt + 1) * 128],
                            wvt[:, kt, :],
                            start=(kt == 0), stop=(kt == 7),
                        )
                    ti = tg * 4 + tt
                    nc.vector.tensor_add(
                        Vg[:, ti, :].rearrange("p (h j) -> p h j", h=HPC)[:, :, 0:64],
                        ps[:].rearrange("p (h j) -> p h j", h=HPC),
                        bv3,
                    )

        # ---------------- attention + interleaved output projection ----------------
        # Heads are processed in PAIRS (2*hi, 2*hi+1): the even head's K/Q
        # rows live in SBUF partitions 0:64, the odd head's in 64:128, so
        # their S matmuls (contraction d_k=64) target disjoint PE row-groups
        # (tile_position auto-derives from base_partition) and run
        # CONCURRENTLY when issued interleaved -- 2x S throughput where a
        # lone d_k=64 matmul leaves half the array idle.
        with ExitStack() as ph:
            wopool = ph.enter_context(tc.tile_pool(name="wopool", bufs=1))
            opool = ph.enter_context(tc.tile_pool(name="opool", bufs=3))
            ppool = ph.enter_context(tc.tile_pool(name="ppool", bufs=20))
            rpool = ph.enter_context(tc.tile_pool(name="rpool", bufs=4))
            s_psum = ph.enter_context(tc.tile_pool(name="s_psum", bufs=2, space="PSUM"))
            op_psum = ph.enter_context(tc.tile_pool(name="op_psum", bufs=2, space="PSUM"))
            pv_psum = ph.enter_context(tc.tile_pool(name="pv_psum", bufs=2, space="PSUM"))

            wot = wopool.tile([128, 4, D], bf16)
            wosrc = wo.rearrange("(i p) n -> p i n", p=128)
            for kt in range(4):
                nc.sync.dma_start(wot[:, kt, :], wosrc[:, kt, :])

            def emit_outproj(tck):
                for e in range(8):
                    ps = op_psum.tile([128, 512], f32, tag="ops")
                    for kt in range(4):
                        nc.tensor.matmul(
                            ps[:],
                            wot[:, kt, e * 128:(e + 1) * 128],
                            xT[:, kt, tck * 512:(tck + 1) * 512],
                            start=(kt == 0), stop=(kt == 3),
                        )
                    ot = opool.tile([128, 512], bf16, tag="otile")
                    nc.vector.tensor_scalar_add(ot[:], ps[:], bo_t[:, e:e + 1])
                    nc.sync.dma_start(
                        outT[e * 128:(e + 1) * 128, tck * 512:(tck + 1) * 512],
                        ot[:],
                    )

            pending_pv = None    # (ptA, ptB, hi, qc): S/exp emitted, PVs pending
            pending_nm = None    # list of (pv, rr, po, hi, q0): normalize pending

            def emit_normalize(pv, rr, po, hi, q0):
                # Replicate 1/d across 64 partitions via PE outer product
                # (this walrus build lacks gpsimd partition_broadcast), then
                # DVE copy + mul (DVE reads only one PSUM operand per inst).
                rp = op_psum.tile([64, 512], f32, tag="ops")
                nc.tensor.matmul(rp[:], ones_t[64:65, :], rr[64:65, :])
                nc.vector.tensor_copy(xT[po:po + 64, hi, q0:q0 + 512], pv[0:64, :])
                nc.vector.tensor_mul(
                    xT[po:po + 64, hi, q0:q0 + 512],
                    xT[po:po + 64, hi, q0:q0 + 512],
                    rp[:],
                )

            def build_pv(ptiles, h, qc):
                """Allocate the PV psum tile and return (normalize-args,
                [mm closures]) so the accumulation chain can be woven between
                the next block's S matmuls (the PE then has independent work
                while ACT drains that block's exp backlog)."""
                po = 64 * (h % 2)
                hi = h // 2
                q0 = qc * 512
                nkt = 4 * qc + 4
                pv = pv_psum.tile([65, 512], f32)

                def mk(kt):
                    pt, off, vs, w = ptiles[kt]

                    def mm():
                        nc.tensor.matmul(
                            pv[:, vs:512],
                            Vg[:, kt, 65 * h:65 * (h + 1)],
                            pt[:, off:off + w],
                            start=(kt == 0), stop=(kt == nkt - 1),
                        )
                    return mm

                def recip():
                    rr_ = rr
                    with nc.allow_low_precision(reason="softmax denom recip in f32r"):
                        if qc < 2 or h % 2 == 1:
                            # ACT ln/exp reciprocal (~1.4us, shares the exp
                            # table set) where the DVE lane-serial reciprocal
                            # (~3.4us) would pace the block; split so neither
                            # engine saturates.
                            srl = rpool.tile([1, 512], f32, tag="srl")
                            nc.scalar.activation(srl[:], pv[64:65, :], LNF)
                            nc.scalar.activation(rr_[64:65, :], srl[:], EXPF, scale=-1.0)
                        else:
                            nc.vector.reciprocal(rr_[64:65, :], pv[64:65, :])

                rr = rpool.tile([65, 512], f32r, tag="rrow")
                mms = [mk(kt) for kt in range(nkt)] + [recip]
                return (pv, rr, po, hi, q0), mms

            def emit_spairs(hi, qc, pvq):
                """S + exp for the head pair (2*hi, 2*hi+1), matmuls issued
                alternating row-halves so the two heads' S tiles run
                concurrently in the PE array. After each [128,1024] S span,
                a few pending PV matmuls from the PREVIOUS block (pvq) are
                woven in: the S spans outrun ACT's exp drain (427ns vs
                1.15us per span warm), and the woven PV chain keeps the PE
                fed instead of stalling on the S psum ring."""
                q0 = qc * 512
                ptA, ptB = [], []

                def weave(n):
                    for _ in range(min(n, len(pvq))):
                        pvq.popleft()()
                # fully-allowed tiles (kt < 4*qc): per-head [128,1024] spans
                # (2 k-tiles), one exp instruction per span
                for m in range(2 * qc):
                    spA = s_psum.tile([128, 1024], f32, tag="spair")
                    ptA2 = ppool.tile([128, 1024], bf16, tag="ppair", bufs=10)
                    spB = s_psum.tile([128, 1024], f32, tag="spair")
                    ptB2 = ppool.tile([128, 1024], bf16, tag="ppair", bufs=10)
                    for half in range(2):
                        kt = 2 * m + half
                        c0 = half * 512
                        nc.tensor.matmul(
                            spA[:, c0:c0 + 512],
                            KT[0:64, hi, kt * 128:(kt + 1) * 128],
                            QT[0:64, hi, q0:q0 + 512],
                        )
                        nc.tensor.matmul(
                            spB[:, c0:c0 + 512],
                            KT[64:128, hi, kt * 128:(kt + 1) * 128],
                            QT[64:128, hi, q0:q0 + 512],
                        )
                    nc.scalar.activation(ptA2[:], spA[:], EXPF, scale=0.125)
                    nc.scalar.activation(ptB2[:], spB[:], EXPF, scale=0.125)
                    weave(3)
                    ptA.append((ptA2, 0, 0, 512))
                    ptA.append((ptA2, 512, 0, 512))
                    ptB.append((ptB2, 0, 0, 512))
                    ptB.append((ptB2, 512, 0, 512))
                # diagonal-region tiles, width-trimmed pairs (512|384) and
                # (256|128) packed per head into one [128,1024] PSUM span
                for dp in range(2):
                    spA = s_psum.tile([128, 1024], f32, tag="spair")
                    ptA2 = ppool.tile([128, 1024], bf16, tag="ppair", bufs=10)
                    spB = s_psum.tile([128, 1024], f32, tag="spair")
                    ptB2 = ppool.tile([128, 1024], bf16, tag="ppair", bufs=10)
                    pair_off = 0
                    for half in range(2):
                        j = 2 * dp + half
                        kt = 4 * qc + j
                        vs = j * 128
                        w = 512 - vs
                        nc.tensor.matmul(
                            spA[:, pair_off:pair_off + w],
                            KT[0:64, hi, kt * 128:(kt + 1) * 128],
                            QT[0:64, hi, q0 + vs:q0 + 512],
                        )
                        nc.tensor.matmul(
                            spB[:, pair_off:pair_off + w],
                            KT[64:128, hi, kt * 128:(kt + 1) * 128],
                            QT[64:128, hi, q0 + vs:q0 + 512],
                        )
                        ptA.append((ptA2, pair_off, vs, w))
                        ptB.append((ptB2, pair_off, vs, w))
                        pair_off += w
                    nc.scalar.activation(
                        ptA2[:, 0:pair_off], spA[:, 0:pair_off],
                        EXPF, scale=0.125,
                    )
                    nc.scalar.activation(
                        ptB2[:, 0:pair_off], spB[:, 0:pair_off],
                        EXPF, scale=0.125,
                    )
                    weave(3)
                    for half in range(2):
                        off = ptA[-2 + half][1]
                        nc.gpsimd.tensor_mul(
                            ptA2[:, off:off + 128], ptA2[:, off:off + 128],
                            tri_t[:],
                        )
                        nc.gpsimd.tensor_mul(
                            ptB2[:, off:off + 128], ptB2[:, off:off + 128],
                            tri_t[:],
                        )
                return ptA, ptB

            # Two-deep software pipeline over the 16 (hi, qc) pair-blocks:
            # block i's PV matmuls are woven between block i+1's S spans,
            # then any remainder is flushed, then block i-1's normalizes.
            # The output projection for group qc is delayed until after the
            # first S-burst of group qc+1 so ACT has exp work while the PE
            # runs the outproj matmuls.
            from collections import deque
            pending_op = None
            for qc in range(4):
                for hi in range(4):
                    pvq = deque()
                    if pending_pv is not None:
                        pA, pB, phi, pqc = pending_pv
                        nmA, amms = build_pv(pA, 2 * phi, pqc)
                        nmB, bmms = build_pv(pB, 2 * phi + 1, pqc)
                        pvq.extend(amms)
                        pvq.extend(bmms)
                    ptA, ptB = emit_spairs(hi, qc, pvq)
                    while pvq:
                        pvq.popleft()()
                    if pending_op is not None and hi == 1:
                        emit_outproj(pending_op)
                        pending_op = None
                    if pending_pv is not None:
                        if pending_nm is not None:
                            for nm in pending_nm:
                                emit_normalize(*nm)
                        pending_nm = [nmA, nmB]
                    pending_pv = (ptA, ptB, hi, qc)

                # group flush: finish every block of this qc so its xT columns
                # are final; the outproj emission is deferred into qc+1
                pA, pB, phi, pqc = pending_pv
                nmA, amms = build_pv(pA, 2 * phi, pqc)
                nmB, bmms = build_pv(pB, 2 * phi + 1, pqc)
                for mm in amms + bmms:
                    mm()
                pending_pv = None
                if pending_nm is not None:
                    for nm in pending_nm:
                        emit_normalize(*nm)
                emit_normalize(*nmA)
                emit_normalize(*nmB)
                pending_nm = None
                pending_op = qc
            emit_outproj(pending_op)


_NC_CACHE = None


def _build_nc():
    global _NC_CACHE
    if _NC_CACHE is None:
        nc = bass.Bass("TRN2", target_bir_lowering=False, debug=False, num_devices=N_CORES)
        _emit_kernel(nc)
        _fixup_sync_waits(nc)
        _NC_CACHE = nc
    return _NC_CACHE


def _host_mask_tiles(attention_mask, key_padding_mask):
    # The kernel exploits the causal structure; verify the runtime masks
    # actually match it (they do for this problem's setup_inputs()).
    am = np.asarray(attention_mask)[0]
    causal = np.triu(np.ones((T, T), np.int32), k=1)
    if not np.array_equal(am != 0, causal != 0):
        raise ValueError("kernel specialised for strict-upper-triangular causal mask")
    if np.asarray(key_padding_mask).any():
        raise ValueError("kernel specialised for all-attendable key_padding_mask")
    # 0/1 multiplicative mask for the 128x128 diagonal block of P^T[k, q]:
    # allowed iff q >= k.
    dk = np.arange(128)[:, None]
    dq = np.arange(128)[None, :]
    return (dq >= dk).astype(np.float32)


def _make_in_maps(inputs):
    import ml_dtypes
    bf = ml_dtypes.bfloat16
    query = np.asarray(inputs["query"], np.float32).astype(bf)
    key = np.asarray(inputs["key"], np.float32).astype(bf)
    value = np.asarray(inputs["value"], np.float32).astype(bf)
    W = {n: np.asarray(inputs[n], np.float32).astype(bf)
         for n in ("W_q", "W_k", "W_v", "W_o")}
    b = {n: np.asarray(inputs[n], np.float32) for n in ("b_q", "b_k", "b_v", "b_o")}
    msk = _host_mask_tiles(inputs["attention_mask"], inputs["key_padding_mask"])

    in_maps = []
    for c in range(N_CORES):
        bb, g = c // 2, c % 2
        hsel = slice(DH * g, DH * (g + 1))
        in_maps.append({
            "qT": np.ascontiguousarray(query[bb].T),
            "kT": np.ascontiguousarray(key[bb].T),
            "vT": np.ascontiguousarray(value[bb].T),
            "wqT": np.ascontiguousarray(W["W_q"].T[:, hsel]),
            "wkT": np.ascontiguousarray(W["W_k"].T[:, hsel]),
            "wvT": np.ascontiguousarray(W["W_v"].T[:, hsel]),
            "woT": np.ascontiguousarray(W["W_o"].T[hsel, :]),
            "bq": np.ascontiguousarray(b["b_q"][hsel].reshape(4, 128).T),
            "bk": np.ascontiguousarray(b["b_k"][hsel].reshape(4, 128).T),
            "bv": np.tile(b["b_v"][hsel][None, :], (128, 1)),
            "bo": np.ascontiguousarray((0.5 * b["b_o"]).reshape(8, 128).T),
            "tri": msk.astype(bf),
        })
    return in_maps


def assemble_output(results):
    """Sum the two per-head-group partial out^T per batch (host-side
    unshard: each core's outT is its head-group's full-D partial)."""
    out = np.empty((B, T, D), np.float32)
    for bb in range(B):
        p0 = np.asarray(results[2 * bb]["outT"], np.float32)
        p1 = np.asarray(results[2 * bb + 1]["outT"], np.float32)
        out[bb] = (p0 + p1).T
    return out


def kernel(**inputs):
    nc = _build_nc()
    in_maps = _make_in_maps(inputs)
    out = None
    for _attempt in range(3):
        res = run_bass_kernel_spmd(nc, in_maps, core_ids=list(range(N_CORES)))
        out = assemble_output(res.results)
        # rare transient device flake can corrupt an execution; retry
        if np.isfinite(out).all():
            break
    return out

